# revision 24
# baseline (speedup 1.0000x reference)
"""AttnBlock (GroupNorm + single-head self-attention + residual) on 8 TRN2 cores.

Strategy: data-parallel over batch (16 images -> 2 per core); no collectives.
Two structural optimizations over a straight port of the reference:

1. Host-side weight fusion. scores = (h Wq)(h Wk)^T = h (Wq^T Wk) h^T, so the
   kernel computes t = h @ M with M = Wq^T Wk precomputed on host (f64) and
   contracts t against h itself -- the entire K projection disappears.
   Generic bias handling survives the fusion: the bq-dependent score term is
   (bq^T Wk) . h_m, folded into t as a per-channel bias; the bk term is
   constant per score row and cancels in softmax; bv contributes Wp @ bv to
   the output (softmax rows sum to 1), folded into bp on host.

2. Everything in bf16. On TRN2 the PE moving-operand stream is byte-rate
   limited (~4 B/cycle/partition), so a bf16 matmul streams 2 columns/cycle --
   N=512 matmuls issue every ~112 ns, the same MAC throughput fp8 DoubleRow
   reaches (PSUM's 512-fp32 bank caps N, so DoubleRow can't go wider), at
   ~0.1% quantization error instead of ~4%. bf16 also gets the fast weight
   load path (FWL), hiding LDWEIGHTS under the matmul stream.

Per-batch dataflow on one core (C=512 channels, N=H*W=1024 tokens):
  x    [C, N]  channel-major (native layout of the input)
  h    = groupnorm(x)  bf16 (stats via bn_stats + tiny matmuls, fp32)
  tT   [c', n] = M.T @ h     (+ bq^T Wk bias)
  sT   [m, n] = h.T @ t   -> exp(s/sqrt(C) - 2) on ACT (shift keeps exp small;
               cancels exactly in softmax)
  den  [1, n] = ones.T @ exp  (PSUM-accumulated over the 8 m-tiles)
  v    [m, c] = h.T @ Wv     (token-major via matmul operand swap)
  ctx  [c, n] = v.T @ exp    scaled by 1/den at evacuation
  yu   [p, n] = Wp.T @ ctx
  out  = x' + yu             (x' = x + bp', prefolded on DVE)
"""

import numpy as np
import ml_dtypes

B, C, HW = 16, 512, 1024
H = W = 32
NCORES = 8
BPC = B // NCORES
GROUPS = 32
GSIZE = C // GROUPS  # 16
EPS = 1e-5
SHIFT = 2.0  # exp(score - SHIFT); cancels in softmax, keeps exp values small

_CACHE = {}


def _build_nc():
    import concourse.bacc as bacc
    import concourse.tile as tile
    from concourse import mybir

    R = mybir.dt.float32r
    F = mybir.dt.float32
    F8 = mybir.dt.float8e4
    BF = mybir.dt.bfloat16
    A = mybir.AluOpType
    AF = mybir.ActivationFunctionType

    nc = bacc.Bacc("TRN2", target_bir_lowering=False, debug=False)

    x = nc.declare_dram_parameter("x", [BPC, C, HW], F, isOutput=False)
    Mf = nc.declare_dram_parameter("Mf", [C, C], BF, isOutput=False)  # Wq^T Wk
    wv = nc.declare_dram_parameter("wv", [C, C], BF, isOutput=False)  # WvT
    wp = nc.declare_dram_parameter("wp", [C, C], BF, isOutput=False)  # WpT
    vecs = nc.declare_dram_parameter("vecs", [128, 4, 5], F, isOutput=False)
    gmask = nc.declare_dram_parameter("gmask", [128, 8], F, isOutput=False)
    gmaskT = nc.declare_dram_parameter("gmaskT", [8, 128], F, isOutput=False)
    ones_row = nc.declare_dram_parameter("ones_row", [1, 128], BF, isOutput=False)
    y = nc.declare_dram_parameter("y", [BPC, C, HW], F, isOutput=True)

    with tile.TileContext(nc) as tc:
        import contextlib

        ctx = contextlib.ExitStack()
        with ctx:
            wpool = ctx.enter_context(tc.tile_pool(name="w", bufs=1))
            cpool = ctx.enter_context(tc.tile_pool(name="c", bufs=1))
            xpool = ctx.enter_context(tc.tile_pool(name="x", bufs=2))
            hpool = ctx.enter_context(tc.tile_pool(name="h", bufs=2))
            qpool = ctx.enter_context(tc.tile_pool(name="q", bufs=2))
            vpool = ctx.enter_context(tc.tile_pool(name="v", bufs=2))
            epool = ctx.enter_context(tc.tile_pool(name="e", bufs=2))
            spool = ctx.enter_context(tc.tile_pool(name="s", bufs=2))
            rpool = ctx.enter_context(tc.tile_pool(name="r", bufs=2))
            opool = ctx.enter_context(tc.tile_pool(name="o", bufs=3))
            mpool = ctx.enter_context(tc.tile_pool(name="mp", bufs=6, space="PSUM"))
            gpool = ctx.enter_context(tc.tile_pool(name="gp", bufs=2, space="PSUM"))

            # ---- persistent loads -------------------------------------------
            # batch-0 x tiles first: the whole pipeline's critical path starts
            # with groupnorm stats, so get those bytes moving before weights.
            xts = []
            for b in range(BPC):
                xt_b = xpool.tile([128, 4, HW], F, tag="x", name=f"xt{b}")
                xts.append(xt_b)
            xsrc = [x.ap()[b].rearrange("(i p) n -> p i n", p=128) for b in range(BPC)]
            from concourse.tile import add_dep_helper

            x0_dmas = []
            for i in range(4):
                for s in range(2):
                    d = nc.sync.dma_start(out=xts[0][:, i, s * 512 : (s + 1) * 512],
                                          in_=xsrc[0][:, i, s * 512 : (s + 1) * 512])
                    x0_dmas.append(d)
            gmask_t = cpool.tile([128, 8], F, tag="gmask")
            nc.sync.dma_start(out=gmask_t, in_=gmask.ap())
            gmaskT_t = cpool.tile([8, 128], F, tag="gmaskT")
            nc.sync.dma_start(out=gmaskT_t, in_=gmaskT.ap())
            vecs_t = cpool.tile([128, 4, 5], F, tag="vecs")
            nc.sync.dma_start(out=vecs_t, in_=vecs.ap())
            ones_row_t = cpool.tile([1, 128], BF, tag="ones_row")
            nc.sync.dma_start(out=ones_row_t, in_=ones_row.ap())
            eps8 = cpool.tile([8, 1], F, tag="eps8")
            nc.vector.memset(eps8, EPS)
            nshift = cpool.tile([128, 1], F, tag="nshift")
            nc.vector.memset(nshift, -SHIFT)
            ones_b = cpool.tile([128, 16], BF, tag="ones_b")
            nc.vector.memset(ones_b, 1.0)

            # PE warmup: keep the tensor engine busy (and HAM un-throttled)
            # while groupnorm stats crunch through the startup latency.
            wrm = cpool.tile([128, 128], F, tag="wrm")
            nc.vector.memset(wrm, 0.0)
            # fp8 probe: measures plain (non-DoubleRow) fp8 matmul issue rate
            # in the trace; harmless zeros accumulating into a scratch bank.
            wrm8 = cpool.tile([128, 512], F8, tag="wrm8")
            nc.vector.memset(wrm8, 0.0)
            s8 = cpool.tile([128, 128], F8, tag="s8")
            nc.vector.memset(s8, 0.0)
            wps = mpool.tile([128, 512], F, tag="mm")

            def warmup(n):
                for j in range(n):
                    nc.tensor.matmul(wps[:, 0:128], wrm, wrm, start=(j == 0),
                                     stop=(j == n - 1))

            warmup(24)
            for j in range(6):
                nc.tensor.matmul(wps, s8, wrm8, start=(j == 0), stop=(j == 5))

            M_t = wpool.tile([128, 4, C], BF, tag="Mf")
            wv_t = wpool.tile([128, 4, C], BF, tag="wv")
            wp_t = wpool.tile([128, 4, C], BF, tag="wp")
            prev = x0_dmas[-1]
            bulk = [(M_t, Mf, None), (wv_t, wv, None), (None, None, 1),
                    (wp_t, wp, None)]
            for t, src, xb in bulk:
                if xb is not None:
                    for i in range(4):
                        d = nc.sync.dma_start(out=xts[xb][:, i, :], in_=xsrc[xb][:, i, :])
                        add_dep_helper(d.ins, prev.ins, reason="dma bandwidth order")
                    prev = d
                else:
                    d = nc.sync.dma_start(
                        out=t, in_=src.ap().rearrange("(ct p) o -> p ct o", p=128))
                    add_dep_helper(d.ins, prev.ins, reason="dma bandwidth order")
                    prev = d

            # ---- groupnorm for both batches, pipelined per 128-channel tile.
            # Each 128-channel tile holds whole groups (16 channels/group), so
            # every tile's normalization chain is independent and unblocks its
            # projection matmuls early.
            hts = []
            for b in range(BPC):
                xt = xts[b]
                ht = hpool.tile([128, 4, HW], BF, tag="hctx", name=f"ht{b}")
                hts.append(ht)
                varga = spool.tile([8, 4], F, tag="varga")
                sda = spool.tile([8, 4], F, tag="sda")
                ggs = {}

                def finish(i, gg, b=b, xt=xt, ht=ht, sda=sda):
                    st2 = spool.tile([8, 2], F, tag=f"st2{i}")
                    with nc.allow_low_precision("groupnorm rstd"):
                        nc.vector.reciprocal(out=st2[:, 0:1], in_=sda[:, i : i + 1])
                    nc.vector.tensor_copy(out=st2[:, 1:2], in_=gg[:, 0:1])
                    bc = gpool.tile([128, 2], F, tag="gn")
                    nc.tensor.matmul(bc, gmaskT_t, st2, start=True, stop=True)
                    scale_c = spool.tile([128, 1], F, tag=f"scale{i}")
                    nc.vector.tensor_mul(out=scale_c, in0=bc[:, 0:1], in1=vecs_t[:, i, 0:1])
                    tmp = spool.tile([128, 1], F, tag=f"tmp{i}")
                    nc.vector.tensor_mul(out=tmp, in0=bc[:, 1:2], in1=scale_c)
                    shift_c = spool.tile([128, 1], F, tag=f"shift{i}")
                    nc.vector.tensor_sub(out=shift_c, in0=vecs_t[:, i, 1:2], in1=tmp)
                    if b == 0 and i < 3:
                        # keep the warmed-up PE fed while the next tile's
                        # groupnorm stats crunch through the vector engine
                        warmup(8 + 2 * i)
                    # batch 0's normalize rides the idle ACT at startup so DVE
                    # can move straight to the next tile's stats; batch 1's
                    # stays on DVE to keep ACT clear for batch 0's exp stream.
                    if b == 0:
                        nc.scalar.activation(out=ht[:, i, :], in_=xt[:, i, :],
                                             func=AF.Identity, bias=shift_c,
                                             scale=scale_c)
                    else:
                        nc.vector.tensor_scalar(
                            out=ht[:, i, :], in0=xt[:, i, :],
                            scalar1=scale_c, scalar2=shift_c, op0=A.mult, op1=A.add)

                for i in range(4):
                    xr = xt[:, i, :].rearrange("p (s d) -> p s d", d=512)
                    st6 = spool.tile([128, 2, 6], F, tag=f"st6{i}")
                    for s in range(2):
                        nc.vector.bn_stats(out=st6[:, s, :], in_=xr[:, s, :])
                    mv = spool.tile([128, 2], F, tag=f"mv{i}")
                    nc.vector.bn_aggr(out=mv, in_=st6)
                    # stats_i = per-channel (mean, E[x^2])
                    stats_i = spool.tile([128, 2], F, tag=f"stats{i}")
                    m2c = spool.tile([128, 1], F, tag=f"m2c{i}")
                    nc.vector.tensor_mul(out=m2c, in0=mv[:, 0:1], in1=mv[:, 0:1])
                    nc.vector.tensor_add(out=stats_i[:, 1:2], in0=mv[:, 1:2], in1=m2c)
                    nc.vector.tensor_copy(out=stats_i[:, 0:1], in_=mv[:, 0:1])
                    gps = gpool.tile([8, 2], F, tag="gn")
                    nc.tensor.matmul(gps, gmask_t, stats_i, start=True, stop=True)
                    # gg = (mean_g, Ex2_g) per group
                    gg = spool.tile([8, 2], F, tag=f"gg{i}")
                    ggs[i] = gg
                    nc.vector.tensor_scalar_mul(out=gg, in0=gps, scalar1=1.0 / GSIZE)
                    m2g = spool.tile([8, 1], F, tag=f"m2g{i}")
                    nc.vector.tensor_mul(out=m2g, in0=gg[:, 0:1], in1=gg[:, 0:1])
                    nc.vector.tensor_sub(out=varga[:, i : i + 1], in0=gg[:, 1:2],
                                         in1=m2g)
                    if b == 0:
                        nc.scalar.activation(out=sda[:, i : i + 1],
                                             in_=varga[:, i : i + 1],
                                             func=AF.Sqrt, bias=eps8, scale=1.0)
                        finish(i, gg)
                if b == 1:
                    nc.scalar.activation(out=sda, in_=varga, func=AF.Sqrt,
                                         bias=eps8, scale=1.0)
                    for i in range(4):
                        finish(i, ggs[i])

            for b in range(BPC):
                xt = xts[b]
                ht = hts[b]
                # ---- t-projection: tT[c', n] = M.T @ h ----------------------
                tb = qpool.tile([128, 4, HW], BF, tag="tb")
                for ot in range(4):
                    pp2 = [mpool.tile([128, 512], F, tag="mm",
                                      name=f"pj{b}_{ot}_{nh}") for nh in range(2)]
                    for ct in range(4):
                        for nh in range(2):
                            nc.tensor.matmul(
                                pp2[nh],
                                M_t[:, ct, ot * 128 : (ot + 1) * 128],
                                ht[:, ct, nh * 512 : (nh + 1) * 512],
                                start=(ct == 0), stop=(ct == 3))
                    for nh in range(2):
                        nc.vector.tensor_scalar_add(
                            out=tb[:, ot, nh * 512 : (nh + 1) * 512],
                            in0=pp2[nh],
                            scalar1=vecs_t[:, ot, 2:3])

                # ---- v-projection: v[m, c] = h.T @ Wv -----------------------
                vt = vpool.tile([128, 8, 512], BF, tag="v")
                for mt in range(8):
                    ps = mpool.tile([128, 512], F, tag="mm")
                    for ct in range(4):
                        nc.tensor.matmul(
                            ps,
                            ht[:, ct, mt * 128 : (mt + 1) * 128],
                            wv_t[:, ct, :],
                            start=(ct == 0), stop=(ct == 3))
                    nc.vector.tensor_copy(out=vt[:, mt, :], in_=ps)

                # x is consumed only by the final residual combine from here
                # on: fold the output-projection bias in now, on DVE (idle
                # during the ACT-bound scores phase), so the tail needs no
                # separate bias op.
                for pt in range(4):
                    nc.vector.tensor_scalar_add(out=xt[:, pt, :], in0=xt[:, pt, :],
                                                scalar1=vecs_t[:, pt, 4:5])

                # ---- scores^T + exp + denominator ---------------------------
                et = epool.tile([128, 8, HW], BF, tag="e")
                psd = [gpool.tile([1, 512], F, tag="gn", name=f"psd{b}_{nh}")
                       for nh in range(2)]
                for mt in range(8):
                    pp2 = [mpool.tile([128, 512], F, tag="mm",
                                      name=f"sc{b}_{mt}_{nh}") for nh in range(2)]
                    for j in range(4):
                        for nh in range(2):
                            nc.tensor.matmul(
                                pp2[nh],
                                ht[:, j, mt * 128 : (mt + 1) * 128],
                                tb[:, j, nh * 512 : (nh + 1) * 512],
                                start=(j == 0), stop=(j == 3))
                    for nh in range(2):
                        nc.scalar.activation(
                            out=et[:, mt, nh * 512 : (nh + 1) * 512], in_=pp2[nh],
                            func=AF.Exp, scale=float(C ** -0.5), bias=nshift)
                    # denominator rides the PE: accumulate ones.T @ exp per
                    # tile as soon as it exists, so the reciprocal chain is
                    # done before ctx evacuation needs it.
                    for nh in range(2):
                        nc.tensor.matmul(
                            psd[nh], ones_b[:, 0:1],
                            et[:, mt, nh * 512 : (nh + 1) * 512],
                            start=(mt == 0), stop=(mt == 7))
                # broadcast first, then reciprocal on all 128 partitions (a
                # [1,512] reciprocal is serial on one partition and ~6x slower
                # than the [128,512] one).
                rc = rpool.tile([1, HW], BF, tag="recip")
                rb_sb = rpool.tile([128, 2, 512], F, tag="rb")
                for nh in range(2):
                    nc.scalar.copy(out=rc[:, nh * 512 : (nh + 1) * 512],
                                   in_=psd[nh])
                    prb = gpool.tile([128, 512], F, tag="gn")
                    nc.tensor.matmul(prb, ones_row_t,
                                     rc[0:1, nh * 512 : (nh + 1) * 512],
                                     start=True, stop=True)
                    # denominators are far from the approximation's edge
                    # cases; its ~2e-6 rel err is noise next to bf16 rounding.
                    nc.vector.reciprocal_approx_fast(out=rb_sb[:, nh, :], in_=prb)

                # ---- context ------------------------------------------------
                ct_t = qpool.tile([128, 4, HW], BF, tag="ct")
                for c2 in range(4):
                    pp2 = [mpool.tile([128, 512], F, tag="mm",
                                      name=f"cx{b}_{c2}_{nh}") for nh in range(2)]
                    for mt in range(8):
                        for nh in range(2):
                            nc.tensor.matmul(
                                pp2[nh],
                                vt[:, mt, c2 * 128 : (c2 + 1) * 128],
                                et[:, mt, nh * 512 : (nh + 1) * 512],
                                start=(mt == 0), stop=(mt == 7))
                    for nh in range(2):
                        nc.vector.tensor_mul(
                            out=ct_t[:, c2, nh * 512 : (nh + 1) * 512],
                            in0=pp2[nh], in1=rb_sb[:, nh, :])

                # ---- output projection + residual ---------------------------
                for pt in range(4):
                    pp2 = [mpool.tile([128, 512], F, tag="mm",
                                      name=f"yp{b}_{pt}_{nh}") for nh in range(2)]
                    for ct in range(4):
                        for nh in range(2):
                            nc.tensor.matmul(
                                pp2[nh],
                                wp_t[:, ct, pt * 128 : (pt + 1) * 128],
                                ct_t[:, ct, nh * 512 : (nh + 1) * 512],
                                start=(ct == 0), stop=(ct == 3))
                    for nh in range(2):
                        o_t = opool.tile([128, 512], F, tag="o1")
                        if nh == 0:
                            nc.vector.tensor_add(
                                out=o_t, in0=pp2[nh],
                                in1=xt[:, pt, nh * 512 : (nh + 1) * 512])
                        else:
                            # split across ACT (PSUM read) + GpSimd (SBUF add)
                            # so the final drain isn't DVE-serial
                            o_s = opool.tile([128, 512], F, tag="o0")
                            nc.scalar.copy(out=o_s, in_=pp2[nh])
                            nc.gpsimd.tensor_add(
                                out=o_t, in0=o_s,
                                in1=xt[:, pt, nh * 512 : (nh + 1) * 512])
                        nc.sync.dma_start(
                            out=y.ap()[b][pt * 128 : (pt + 1) * 128, nh * 512 : (nh + 1) * 512],
                            in_=o_t)

    nc.finalize()
    return nc


def _get_nc():
    if "nc" not in _CACHE:
        _CACHE["nc"] = _build_nc()
    return _CACHE["nc"]


def make_in_maps(inputs):
    x = np.asarray(inputs["x"], np.float32).reshape(B, C, HW)
    f32 = lambda a: np.ascontiguousarray(np.asarray(a, np.float32))
    f64 = lambda a: np.asarray(a, np.float64)
    BF = ml_dtypes.bfloat16

    # scores fusion: M = Wq^T Wk  (contraction c x c'); t-bias = bq^T Wk
    Mf = np.ascontiguousarray(
        (f64(inputs["wq"]).T @ f64(inputs["wk"])).astype(BF))
    tbias = (f64(inputs["bq"]) @ f64(inputs["wk"])).astype(np.float32)
    wvT = np.ascontiguousarray(f32(inputs["wv"]).T.astype(BF))
    wpT = np.ascontiguousarray(f32(inputs["wp"]).T.astype(BF))
    # bv contributes Wp @ bv to the output (attention rows sum to 1)
    bp_f = (f64(inputs["bp"]) + f64(inputs["wp"]) @ f64(inputs["bv"])).astype(np.float32)

    vstack = np.stack([f32(inputs["gn_w"]), f32(inputs["gn_b"]), tbias,
                       np.zeros(C, np.float32), bp_f])  # [5, C]
    # vecs[p, i, v] = vstack[v, i*128 + p]
    vecs = np.ascontiguousarray(vstack.reshape(5, 4, 128).transpose(2, 1, 0))
    gmask = np.zeros((128, 8), np.float32)
    for p in range(128):
        gmask[p, p // GSIZE] = 1.0
    gmaskT = gmask.T.copy()
    ones_row = np.ones((1, 128), BF)

    shared = {"Mf": Mf, "wv": wvT, "wp": wpT, "vecs": vecs,
              "gmask": gmask, "gmaskT": gmaskT, "ones_row": ones_row}
    return [dict(shared, x=np.ascontiguousarray(x[i * BPC : (i + 1) * BPC]))
            for i in range(NCORES)]


def kernel(**inputs) -> np.ndarray:
    from concourse.bass_utils import run_bass_kernel_spmd

    core_ids = list(range(NCORES))
    in_maps = make_in_maps(inputs)
    nc = _get_nc()
    res = run_bass_kernel_spmd(nc, in_maps, core_ids)
    out = np.concatenate([res.results[i]["y"] for i in core_ids], axis=0)
    return out.reshape(B, C, H, W)


# revision 25
# speedup vs baseline: 1.3516x; 1.3516x over previous
"""AttnBlock (GroupNorm + single-head self-attention + residual) on 8 TRN2 cores.

Strategy: data-parallel over batch (16 images -> 2 per core); no collectives.
Two structural optimizations over a straight port of the reference:

1. Host-side weight fusion. scores = (h Wq)(h Wk)^T = h (Wq^T Wk) h^T, so the
   kernel computes t = h @ M with M = Wq^T Wk precomputed on host (f64) and
   contracts t against h itself -- the entire K projection disappears.
   Generic bias handling survives the fusion: the bq-dependent score term is
   (bq^T Wk) . h_m, folded into t as a per-channel bias; the bk term is
   constant per score row and cancels in softmax; bv contributes Wp @ bv to
   the output (softmax rows sum to 1), folded into bp on host.

2. fp8 (e4m3) DoubleRow matmuls where quantization noise is provably cheap
   (verified against the oracle offline): scores (h8 x t8), v-proj (h8 x Wv8),
   ctx (v8 x exp8) and out-proj (Wp8 x ctx8). On TRN2 every non-fp32 matmul
   streams 1 column/cycle (N=512 -> ~216 ns warm) regardless of dtype, and
   PSUM's 512-fp32 bank caps N, so DoubleRow's 2-weights/cell is the ONLY
   mode that doubles MAC throughput. The t-projection stays bf16 (full
   precision-ish) since quantizing its operands pushes total error too close
   to the 2e-2 gate. Scale management: Wv/Wp pre-scaled by 64 on host (fp8
   resolution), ctx scaled by 32 at evacuation (via the folded softmax
   reciprocal); the final 1/(32*64) rides the fused tail op.

Per-batch dataflow on one core (C=512 channels, N=H*W=1024 tokens):
  x    [C, N]  channel-major (native layout of the input)
  h    = groupnorm(x)  bf16 + fp8 copies (stats via bn_stats + tiny matmuls)
  tT   [c', n] = M.T @ h   bf16, 4 c-tiles in PSUM  -> t8 (+ bq^T Wk bias)
  v    [m, c] = h8.T @ Wv8   DoubleRow, token-major   -> v8 / 64
  sT   [m, n] = h8.T @ t8    DoubleRow -> exp(s/sqrt(C) - 2) on ACT
               (constant shift keeps exp in fp8 range; cancels in softmax)
  den  [1, n] = ones.T @ exp  DoubleRow (PSUM-accumulated)
  ctx  [c, n] = v8.T @ exp8  DoubleRow, scaled by 32/den at evacuation -> c8
  yu   [p, n] = Wp8.T @ c8   DoubleRow
  out  = x' + yu / 2048      (x' = x + bp', prefolded; single fused DVE op)
"""

import numpy as np
import ml_dtypes

B, C, HW = 16, 512, 1024
H = W = 32
NCORES = 8
BPC = B // NCORES
GROUPS = 32
GSIZE = C // GROUPS  # 16
EPS = 1e-5
SHIFT = 2.0       # exp(score - SHIFT): keeps exp <= ~120 < 240 (fp8e4 max)
S_W = 64.0        # host pre-scale on Wv, Wp before fp8 quantization
S_CTX = 32.0      # ctx scale applied via the softmax reciprocal broadcast

_CACHE = {}


def _build_nc():
    import concourse.bacc as bacc
    import concourse.tile as tile
    from concourse import mybir

    R = mybir.dt.float32r
    F = mybir.dt.float32
    F8 = mybir.dt.float8e4
    BF = mybir.dt.bfloat16
    DR = mybir.MatmulPerfMode.DoubleRow
    A = mybir.AluOpType
    AF = mybir.ActivationFunctionType

    nc = bacc.Bacc("TRN2", target_bir_lowering=False, debug=False)

    x = nc.declare_dram_parameter("x", [BPC, C, HW], F, isOutput=False)
    Mf = nc.declare_dram_parameter("Mf", [C, C], BF, isOutput=False)  # Wq^T Wk
    wv = nc.declare_dram_parameter("wv", [C, C], F8, isOutput=False)  # WvT * 64
    wp = nc.declare_dram_parameter("wp", [C, C], F8, isOutput=False)  # WpT * 64
    vecs = nc.declare_dram_parameter("vecs", [128, 4, 5], F, isOutput=False)
    gmask = nc.declare_dram_parameter("gmask", [128, 8], F, isOutput=False)
    gmaskT = nc.declare_dram_parameter("gmaskT", [8, 128], F, isOutput=False)
    ones_row = nc.declare_dram_parameter("ones_row", [1, 128], R, isOutput=False)
    y = nc.declare_dram_parameter("y", [BPC, C, HW], F, isOutput=True)

    with tile.TileContext(nc) as tc:
        import contextlib

        ctx = contextlib.ExitStack()
        with ctx:
            wpool = ctx.enter_context(tc.tile_pool(name="w", bufs=1))
            cpool = ctx.enter_context(tc.tile_pool(name="c", bufs=1))
            xpool = ctx.enter_context(tc.tile_pool(name="x", bufs=2))
            hpool = ctx.enter_context(tc.tile_pool(name="h", bufs=2))
            h8pool = ctx.enter_context(tc.tile_pool(name="h8", bufs=2))
            qpool = ctx.enter_context(tc.tile_pool(name="q", bufs=2))
            vpool = ctx.enter_context(tc.tile_pool(name="v", bufs=2))
            epool = ctx.enter_context(tc.tile_pool(name="e", bufs=2))
            spool = ctx.enter_context(tc.tile_pool(name="s", bufs=2))
            rpool = ctx.enter_context(tc.tile_pool(name="r", bufs=2))
            opool = ctx.enter_context(tc.tile_pool(name="o", bufs=3))
            mpool = ctx.enter_context(tc.tile_pool(name="mp", bufs=6, space="PSUM"))
            gpool = ctx.enter_context(tc.tile_pool(name="gp", bufs=2, space="PSUM"))

            # ---- persistent loads -------------------------------------------
            # batch-0 x tiles first: the whole pipeline's critical path starts
            # with groupnorm stats, so get those bytes moving before weights.
            xts = []
            for b in range(BPC):
                xt_b = xpool.tile([128, 4, HW], F, tag="x", name=f"xt{b}")
                xts.append(xt_b)
            xsrc = [x.ap()[b].rearrange("(i p) n -> p i n", p=128) for b in range(BPC)]
            from concourse.tile import add_dep_helper

            x0_dmas = []
            for i in range(4):
                for s in range(2):
                    d = nc.sync.dma_start(out=xts[0][:, i, s * 512 : (s + 1) * 512],
                                          in_=xsrc[0][:, i, s * 512 : (s + 1) * 512])
                    x0_dmas.append(d)
            gmask_t = cpool.tile([128, 8], F, tag="gmask")
            nc.sync.dma_start(out=gmask_t, in_=gmask.ap())
            gmaskT_t = cpool.tile([8, 128], F, tag="gmaskT")
            nc.sync.dma_start(out=gmaskT_t, in_=gmaskT.ap())
            vecs_t = cpool.tile([128, 4, 5], F, tag="vecs")
            nc.sync.dma_start(out=vecs_t, in_=vecs.ap())
            ones_row_t = cpool.tile([1, 128], R, tag="ones_row")
            nc.sync.dma_start(out=ones_row_t, in_=ones_row.ap())
            eps8 = cpool.tile([8, 1], F, tag="eps8")
            nc.vector.memset(eps8, EPS)
            nshift = cpool.tile([128, 1], F, tag="nshift")
            nc.vector.memset(nshift, -SHIFT)
            # [128, 2, 16] so the DoubleRow weight AP's k-tile stride is 16 B
            # (s3 dual-fp8 LDWEIGHTS requires step % 16 == 0); only [:, :, 0:1]
            # is used as the ones column.
            ones2 = cpool.tile([128, 2, 16], F8, tag="ones2")
            nc.vector.memset(ones2, 1.0)

            # PE warmup: keep the tensor engine busy (and HAM un-throttled)
            # while groupnorm stats crunch through the startup latency.
            wrm = cpool.tile([128, 128], F, tag="wrm")
            nc.vector.memset(wrm, 0.0)
            wps = mpool.tile([128, 512], F, tag="mm")

            def warmup(n):
                for j in range(n):
                    nc.tensor.matmul(wps[:, 0:128], wrm, wrm, start=(j == 0),
                                     stop=(j == n - 1))

            warmup(24)

            M_t = wpool.tile([128, 4, C], BF, tag="Mf")
            wv_t = wpool.tile([128, 4, C], F8, tag="wv")
            wp_t = wpool.tile([128, 4, C], F8, tag="wp")
            prev = x0_dmas[-1]
            bulk = [(M_t, Mf, None), (wv_t, wv, None), (None, None, 1),
                    (wp_t, wp, None)]
            for t, src, xb in bulk:
                if xb is not None:
                    for i in range(4):
                        d = nc.sync.dma_start(out=xts[xb][:, i, :], in_=xsrc[xb][:, i, :])
                        add_dep_helper(d.ins, prev.ins, reason="dma bandwidth order")
                    prev = d
                else:
                    d = nc.sync.dma_start(
                        out=t, in_=src.ap().rearrange("(ct p) o -> p ct o", p=128))
                    add_dep_helper(d.ins, prev.ins, reason="dma bandwidth order")
                    prev = d

            # ---- groupnorm for both batches, pipelined per 128-channel tile.
            # Each 128-channel tile holds whole groups (16 channels/group), so
            # every tile's normalization chain is independent. The normalize is
            # written twice -- bf16 (t-projection) and fp8 (everything else) --
            # from xt on different engines so neither copy serializes.
            hts = []
            h8s = []
            for b in range(BPC):
                xt = xts[b]
                ht = hpool.tile([128, 4, HW], BF, tag="hctx", name=f"ht{b}")
                hts.append(ht)
                h8 = h8pool.tile([128, 4, HW], F8, tag="h8", name=f"h8{b}")
                h8s.append(h8)
                varga = spool.tile([8, 4], F, tag="varga")
                sda = spool.tile([8, 4], F, tag="sda")
                ggs = {}

                def finish(i, gg, b=b, xt=xt, ht=ht, h8=h8, sda=sda):
                    st2 = spool.tile([8, 2], F, tag=f"st2{i}")
                    with nc.allow_low_precision("groupnorm rstd"):
                        nc.vector.reciprocal(out=st2[:, 0:1], in_=sda[:, i : i + 1])
                    nc.vector.tensor_copy(out=st2[:, 1:2], in_=gg[:, 0:1])
                    bc = gpool.tile([128, 2], F, tag="gn")
                    nc.tensor.matmul(bc, gmaskT_t, st2, start=True, stop=True)
                    scale_c = spool.tile([128, 1], F, tag=f"scale{i}")
                    nc.vector.tensor_mul(out=scale_c, in0=bc[:, 0:1], in1=vecs_t[:, i, 0:1])
                    tmp = spool.tile([128, 1], F, tag=f"tmp{i}")
                    nc.vector.tensor_mul(out=tmp, in0=bc[:, 1:2], in1=scale_c)
                    shift_c = spool.tile([128, 1], F, tag=f"shift{i}")
                    nc.vector.tensor_sub(out=shift_c, in0=vecs_t[:, i, 1:2], in1=tmp)
                    if b == 0 and i < 3:
                        # keep the warmed-up PE fed while the next tile's
                        # groupnorm stats crunch through the vector engine
                        warmup(8 + 2 * i)
                    # three engines, all reading xt directly: ACT/DVE write the
                    # bf16 copy (swapped per batch), GpSimd -- otherwise idle --
                    # always writes the fp8 copy.
                    if b == 0:
                        nc.scalar.activation(out=ht[:, i, :], in_=xt[:, i, :],
                                             func=AF.Identity, bias=shift_c,
                                             scale=scale_c)
                    else:
                        nc.vector.tensor_scalar(
                            out=ht[:, i, :], in0=xt[:, i, :],
                            scalar1=scale_c, scalar2=shift_c, op0=A.mult, op1=A.add)
                    nc.gpsimd.tensor_scalar(
                        out=h8[:, i, :], in0=xt[:, i, :],
                        scalar1=scale_c, scalar2=shift_c, op0=A.mult, op1=A.add)

                for i in range(4):
                    xr = xt[:, i, :].rearrange("p (s d) -> p s d", d=512)
                    st6 = spool.tile([128, 2, 6], F, tag=f"st6{i}")
                    for s in range(2):
                        nc.vector.bn_stats(out=st6[:, s, :], in_=xr[:, s, :])
                    mv = spool.tile([128, 2], F, tag=f"mv{i}")
                    nc.vector.bn_aggr(out=mv, in_=st6)
                    # stats_i = per-channel (mean, E[x^2])
                    stats_i = spool.tile([128, 2], F, tag=f"stats{i}")
                    m2c = spool.tile([128, 1], F, tag=f"m2c{i}")
                    nc.vector.tensor_mul(out=m2c, in0=mv[:, 0:1], in1=mv[:, 0:1])
                    nc.vector.tensor_add(out=stats_i[:, 1:2], in0=mv[:, 1:2], in1=m2c)
                    nc.vector.tensor_copy(out=stats_i[:, 0:1], in_=mv[:, 0:1])
                    gps = gpool.tile([8, 2], F, tag="gn")
                    nc.tensor.matmul(gps, gmask_t, stats_i, start=True, stop=True)
                    # gg = (mean_g, Ex2_g) per group
                    gg = spool.tile([8, 2], F, tag=f"gg{i}")
                    ggs[i] = gg
                    nc.vector.tensor_scalar_mul(out=gg, in0=gps, scalar1=1.0 / GSIZE)
                    m2g = spool.tile([8, 1], F, tag=f"m2g{i}")
                    nc.vector.tensor_mul(out=m2g, in0=gg[:, 0:1], in1=gg[:, 0:1])
                    nc.vector.tensor_sub(out=varga[:, i : i + 1], in0=gg[:, 1:2],
                                         in1=m2g)
                    if b == 0:
                        nc.scalar.activation(out=sda[:, i : i + 1],
                                             in_=varga[:, i : i + 1],
                                             func=AF.Sqrt, bias=eps8, scale=1.0)
                        finish(i, gg)
                if b == 1:
                    nc.scalar.activation(out=sda, in_=varga, func=AF.Sqrt,
                                         bias=eps8, scale=1.0)
                    for i in range(4):
                        finish(i, ggs[i])

            for b in range(BPC):
                xt = xts[b]
                ht = hts[b]
                h8 = h8s[b]
                # ---- t-projection (bf16): tT[c', n] = M.T @ h ---------------
                t8 = qpool.tile([128, 4, HW], F8, tag="t8")
                for ot in range(4):
                    pp2 = [mpool.tile([128, 512], F, tag="mm",
                                      name=f"pj{b}_{ot}_{nh}") for nh in range(2)]
                    for ct in range(4):
                        for nh in range(2):
                            nc.tensor.matmul(
                                pp2[nh],
                                M_t[:, ct, ot * 128 : (ot + 1) * 128],
                                ht[:, ct, nh * 512 : (nh + 1) * 512],
                                start=(ct == 0), stop=(ct == 3))
                    for nh in range(2):
                        nc.vector.tensor_scalar_add(
                            out=t8[:, ot, nh * 512 : (nh + 1) * 512],
                            in0=pp2[nh],
                            scalar1=vecs_t[:, ot, 2:3])

                # ---- v-projection (DoubleRow): v[m, c] = h8.T @ Wv8 ---------
                vt = vpool.tile([128, 8, 512], F8, tag="v")
                for mt in range(8):
                    ps = mpool.tile([128, 512], F, tag="mm")
                    for i2 in range(2):
                        nc.tensor.matmul(
                            ps,
                            h8[:, 2 * i2 : 2 * i2 + 2, mt * 128 : (mt + 1) * 128],
                            wv_t[:, 2 * i2 : 2 * i2 + 2, :],
                            start=(i2 == 0), stop=(i2 == 1), perf_mode=DR)
                    nc.scalar.mul(out=vt[:, mt, :], in_=ps, mul=1.0 / S_W)

                # x is consumed only by the final residual combine from here
                # on: fold the output-projection bias in now so the tail is a
                # single fused DVE op per tile.
                for pt in range(4):
                    nc.scalar.activation(out=xt[:, pt, :], in_=xt[:, pt, :],
                                         func=AF.Identity,
                                         bias=vecs_t[:, pt, 4:5], scale=1.0)

                # ---- scores^T + exp (DoubleRow) -----------------------------
                et = epool.tile([128, 8, HW], F8, tag="e")
                psd = [gpool.tile([1, 512], F, tag="gn", name=f"psd{b}_{nh}")
                       for nh in range(2)]
                for mt in range(8):
                    pp2 = [mpool.tile([128, 512], F, tag="mm",
                                      name=f"sc{b}_{mt}_{nh}") for nh in range(2)]
                    for j2 in range(2):
                        for nh in range(2):
                            nc.tensor.matmul(
                                pp2[nh],
                                h8[:, 2 * j2 : 2 * j2 + 2, mt * 128 : (mt + 1) * 128],
                                t8[:, 2 * j2 : 2 * j2 + 2, nh * 512 : (nh + 1) * 512],
                                start=(j2 == 0), stop=(j2 == 1), perf_mode=DR)
                    for nh in range(2):
                        nc.scalar.activation(
                            out=et[:, mt, nh * 512 : (nh + 1) * 512], in_=pp2[nh],
                            func=AF.Exp, scale=float(C ** -0.5), bias=nshift)
                # softmax denominator: DoubleRow ones-matmuls after the whole
                # scores stream (interleaving them per-tile makes the PE wait
                # on each ACT exp evacuation; batched here it waits once).
                for nh in range(2):
                    for g in range(4):
                        nc.tensor.matmul(
                            psd[nh], ones2[:, :, 0:1],
                            et[:, 2 * g : 2 * g + 2, nh * 512 : (nh + 1) * 512],
                            start=(g == 0), stop=(g == 3), perf_mode=DR)
                # broadcast first, then reciprocal on all 128 partitions (a
                # [1,512] reciprocal is serial on one partition and ~6x slower
                # than the [128,512] one). rb = S_CTX / den (the 1/S_CTX lives
                # in ones_row), folded into the ctx evacuation so ct8 lands at
                # fp8-friendly scale.
                rc = rpool.tile([1, HW], R, tag="recip")
                rb_sb = rpool.tile([128, 2, 512], F, tag="rb")
                for nh in range(2):
                    nc.scalar.copy(out=rc[:, nh * 512 : (nh + 1) * 512],
                                   in_=psd[nh])
                    prb = gpool.tile([128, 512], F, tag="gn")
                    nc.tensor.matmul(prb, ones_row_t,
                                     rc[0:1, nh * 512 : (nh + 1) * 512],
                                     start=True, stop=True)
                    # denominators are far from the approximation's edge
                    # cases; its ~2e-6 rel err is noise next to the fp8
                    # quantization.
                    nc.vector.reciprocal_approx_fast(out=rb_sb[:, nh, :], in_=prb)

                # ---- context (DoubleRow) ------------------------------------
                ct_t = qpool.tile([128, 4, HW], F8, tag="ct")
                for c2 in range(4):
                    pp2 = [mpool.tile([128, 512], F, tag="mm",
                                      name=f"cx{b}_{c2}_{nh}") for nh in range(2)]
                    for g2 in range(4):
                        for nh in range(2):
                            nc.tensor.matmul(
                                pp2[nh],
                                vt[:, 2 * g2 : 2 * g2 + 2, c2 * 128 : (c2 + 1) * 128],
                                et[:, 2 * g2 : 2 * g2 + 2, nh * 512 : (nh + 1) * 512],
                                start=(g2 == 0), stop=(g2 == 3), perf_mode=DR)
                    for nh in range(2):
                        nc.vector.tensor_mul(
                            out=ct_t[:, c2, nh * 512 : (nh + 1) * 512],
                            in0=pp2[nh], in1=rb_sb[:, nh, :])

                # ---- output projection (DoubleRow) + residual ---------------
                # psum = 32*64*(ctx @ Wp); bp' was prefolded into xt, so the
                # tail is one fused DVE op: out = psum/2048 + x'.
                for pt in range(4):
                    pp2 = [mpool.tile([128, 512], F, tag="mm",
                                      name=f"yp{b}_{pt}_{nh}") for nh in range(2)]
                    for i2 in range(2):
                        for nh in range(2):
                            nc.tensor.matmul(
                                pp2[nh],
                                wp_t[:, 2 * i2 : 2 * i2 + 2, pt * 128 : (pt + 1) * 128],
                                ct_t[:, 2 * i2 : 2 * i2 + 2, nh * 512 : (nh + 1) * 512],
                                start=(i2 == 0), stop=(i2 == 1), perf_mode=DR)
                    for nh in range(2):
                        o_t = opool.tile([128, 512], F, tag="o1")
                        nc.vector.scalar_tensor_tensor(
                            out=o_t, in0=pp2[nh], scalar=1.0 / (S_W * S_CTX),
                            in1=xt[:, pt, nh * 512 : (nh + 1) * 512],
                            op0=A.mult, op1=A.add)
                        nc.sync.dma_start(
                            out=y.ap()[b][pt * 128 : (pt + 1) * 128, nh * 512 : (nh + 1) * 512],
                            in_=o_t)

    nc.finalize()
    return nc


def _get_nc():
    if "nc" not in _CACHE:
        _CACHE["nc"] = _build_nc()
    return _CACHE["nc"]


def make_in_maps(inputs):
    x = np.asarray(inputs["x"], np.float32).reshape(B, C, HW)
    f32 = lambda a: np.ascontiguousarray(np.asarray(a, np.float32))
    f64 = lambda a: np.asarray(a, np.float64)
    E4 = ml_dtypes.float8_e4m3

    # scores fusion: M = Wq^T Wk  (contraction c x c'); t-bias = bq^T Wk
    Mf = np.ascontiguousarray(
        (f64(inputs["wq"]).T @ f64(inputs["wk"])).astype(ml_dtypes.bfloat16))
    tbias = (f64(inputs["bq"]) @ f64(inputs["wk"])).astype(np.float32)
    # v/out side: WvT, WpT pre-scaled for fp8; bv folded into bp via Wp
    wvT8 = np.ascontiguousarray((f32(inputs["wv"]).T * S_W).astype(E4))
    wpT8 = np.ascontiguousarray((f32(inputs["wp"]).T * S_W).astype(E4))
    bp_f = (f64(inputs["bp"]) + f64(inputs["wp"]) @ f64(inputs["bv"])).astype(np.float32)

    vstack = np.stack([f32(inputs["gn_w"]), f32(inputs["gn_b"]), tbias,
                       np.zeros(C, np.float32), bp_f])  # [5, C]
    # vecs[p, i, v] = vstack[v, i*128 + p]
    vecs = np.ascontiguousarray(vstack.reshape(5, 4, 128).transpose(2, 1, 0))
    gmask = np.zeros((128, 8), np.float32)
    for p in range(128):
        gmask[p, p // GSIZE] = 1.0
    gmaskT = gmask.T.copy()
    ones_row = np.full((1, 128), 1.0 / S_CTX, np.float32)

    shared = {"Mf": Mf, "wv": wvT8, "wp": wpT8, "vecs": vecs,
              "gmask": gmask, "gmaskT": gmaskT, "ones_row": ones_row}
    return [dict(shared, x=np.ascontiguousarray(x[i * BPC : (i + 1) * BPC]))
            for i in range(NCORES)]


def kernel(**inputs) -> np.ndarray:
    from concourse.bass_utils import run_bass_kernel_spmd

    core_ids = list(range(NCORES))
    in_maps = make_in_maps(inputs)
    nc = _get_nc()
    res = run_bass_kernel_spmd(nc, in_maps, core_ids)
    out = np.concatenate([res.results[i]["y"] for i in core_ids], axis=0)
    return out.reshape(B, C, H, W)
